# revision 41
# baseline (speedup 1.0000x reference)
"""Trainium2 Bass kernel for nn_Autorec_DG_13116830122688 (AutoRec + GraphConv0D).

Math (reference):
    h   = sigmoid(x @ enc_w.T + enc_b)                      [N, 500]
    agg = segment_sum(h[src] * edge_weight, dst, N)
    hm  = conv_w * agg + (1 - conv_w) * h
    p   = clip(hm @ dec_w.T + dec_b, 1, 5)
    p   = where(ft_n0 == 0 rows, fill, p); where(ft_n1 == 0 cols, fill, p)

Strategy (8 NeuronCores, data-parallel over users):
  - Shard users 2500/core (padded to 2560 = 20x128 tiles).
  - Encoder: x is pre-transposed to item-major [128, KC*128] tiles ON HOST
    (bf16), so each user tile is one contiguous 1.5MB DMA and the 47-chunk
    matmul accumulation runs with no PE transposes.  Encoder bias folded in
    as an extra always-one input column.  ACT sigmoid -> h bf16 (SBUF
    resident for the whole kernel).  Decoder weights and gather indices are
    loaded AFTER the first x tile so the PE starts ~35us earlier.
  - AllGather h (bf16, only the 2500 real rows, 512-wide) in 6 chunks
    overlapped with the encoder so every core can gather any source
    embedding; first chunk after 3 tiles keeps the serial collective stream
    ahead of the encoder, small last chunk keeps the exposed tail short.
  - Message passing: edges are filtered (masked-dst rows dropped), scaled by
    conv_w, self-loops with weight (1-conv_w) added, sorted by dst and packed
    into 128-edge blocks per 128-dst tile.  ONE gpsimd dma_gather per dst
    tile fetches all its source rows (sub-1us issue; int16 indices in the
    16-partition wrapped layout), then each block multiplies a host-built
    [128 edges x 128 dst] sparse weight matrix on the TensorEngine:
    aggT += G.T @ W accumulates in PSUM in hidden-major layout, which feeds
    the decoder with no extra transpose.  The self-loop block reads this
    core's h directly from SBUF (no DMA).
  - Decoder: p = hmT.T @ dec_w.T with the column mask and fill constant baked
    into host-prepped weights, plus two extra hidden units carrying the decoder
    bias and the row-mask fill. Single DVE instruction clips to [1, 5] and
    emits fp16 (upcast to f32 on host).  Decoder of tile t-1 is emitted after
    message matmuls of tile t so the PE never idles waiting on the hmT copy.
"""

import os
import sys

import numpy as np

for _p in ("/opt/trn_rl_repo",):
    if _p not in sys.path and os.path.isdir(_p):
        sys.path.insert(0, _p)

import ml_dtypes  # noqa: E402

# ---- problem constants (hardcoded per contest rules) ----
N_USERS = 20000
N_ITEMS = 6000
HIDDEN = 500
M = 8  # cores
UPC = N_USERS // M  # 2500 users per core
UT = 20  # user tiles per core
UPAD = UT * 128  # 2560
KC = 47  # item chunks of 128 (6016 = 47*128 >= 6001 incl. bias col)
IPAD = KC * 128  # 6016
HPAD = 504  # hidden padded: 4 chunks of 126 (500 real + bias/mask units)
NCH = 12  # decoder output chunks of 500 (12*500 = 6000)
R_MIN, R_MAX = 1.0, 5.0
# all-gather chunk boundaries in user tiles (cumulative).  The serial
# collective stream lags the encoder by (first-chunk ready + trigger setup)
# and catches up only ~1-2us per MB, so a 1-tile first chunk starts the
# stream as early as the init barrier allows and the stream reaches the
# final chunk ready-gated instead of stream-gated.
CC_TILE_BOUNDS = [1, 5, 9, 13, 17, 20]
# number of "primed" dst tiles whose epoch-A message pass runs during the
# all-gather tail.  Measured on hardware: the prime's prefix gathers steal
# interconnect bandwidth from the in-flight all-gather chunks and push the
# collective's completion out by as much PE time as they fill, so it nets
# zero at best.  Disabled.
F_PRIME = 0

_bf16 = ml_dtypes.bfloat16

_PROGRAM_CACHE = {}


def _build_program(S_A, S_B, S_U):
    """Build the SPMD Bass program.

    The first F = len(S_A) dst tiles are "primed": their message matmuls are
    split into epoch A (sources in the h_full prefix covered by the first
    EPC all-gather chunks, plus the self-loop) and epoch B (sources in the
    suffix).  Epoch A runs during the all-gather tail, its partial agg is
    parked in SBUF f32 and added back into epoch B's PSUM group via an exact
    f32 identity matmul.  Tiles F..UT-1 use a single unified pass (S_U).
    """
    import concourse.bass as bass
    import concourse.bacc as bacc
    import concourse.mybir as mybir
    from concourse import library_config
    from concourse.masks import make_identity
    from concourse.tile import TileContext

    P = 128
    f32 = mybir.dt.float32
    f16 = mybir.dt.float16
    bf16 = mybir.dt.bfloat16
    F = len(S_A)
    NBLK_A = sum(S_A)
    NBLK_B = sum(S_B)
    NBLK = NBLK_A + NBLK_B + sum(S_U)
    BOFF_A = [sum(S_A[:t]) for t in range(F)]
    BOFF_B = [NBLK_A + sum(S_B[:t]) for t in range(F)]
    BOFF_U = [NBLK_A + NBLK_B + sum(S_U[:t]) for t in range(UT - F)]

    nc = bacc.Bacc(
        "TRN2",
        target_bir_lowering=False,
        debug=False,
        num_devices=M,
        num_swdge_queues=4,
    )

    # x pre-transposed on host: row ut*128+p (item-in-chunk), col k*128+u
    x_d = nc.declare_dram_parameter("x", [UPAD, KC * P], bf16, isOutput=False)
    encw_d = nc.declare_dram_parameter("encw", [P, KC * HIDDEN], bf16, isOutput=False)
    decw_d = nc.declare_dram_parameter("decw", [P, 4 * N_ITEMS], bf16, isOutput=False)
    i16 = mybir.dt.int16
    # gather indices for dma_gather: idx j of tile t at column boff[t]*8 +
    # j//16, partition j%16, replicated 8x down the 128 partitions.
    si_d = nc.declare_dram_parameter("sidx", [P, NBLK * 8], i16, isOutput=False)
    wb_d = nc.declare_dram_parameter("wblk", [NBLK + UT, P, P], bf16, isOutput=False)
    rv_d = nc.declare_dram_parameter("rowvec", [4, UPAD], bf16, isOutput=False)
    out_d = nc.declare_dram_parameter("out", [UPC, N_ITEMS], f16, isOutput=True)

    # only the 2500 real rows travel through the all-gather.  Rows are 512
    # wide (dma_gather needs a 256B-multiple row stride); cols 500-511 are
    # never written — the garbage lands only in hmT rows that the rv DMA
    # overwrites (hidden 500+) or that carry zero decoder weight.
    HW = 512
    h_loc = nc.dram_tensor("h_loc", [UPC, HW], bf16)
    h_full = nc.dram_tensor("h_full", [M * UPC, HW], bf16, addr_space="Shared")
    CC_ROW_BOUNDS = [min(b * 128, UPC) for b in CC_TILE_BOUNDS]
    EPC = 4  # all-gather chunks forming the h_full prefix (epoch A)
    PFX = M * CC_ROW_BOUNDS[EPC - 1] * 1  # prefix rows: chunk-major layout

    with TileContext(nc) as tc:
        with (
            tc.tile_pool(name="const", bufs=1) as cpool,
            tc.tile_pool(name="xin", bufs=2) as xpool,
            tc.tile_pool(name="hsb", bufs=UT) as hpool,
            tc.tile_pool(name="gat", bufs=3 if F > 0 else 4) as gpool,
            tc.tile_pool(name="wbl", bufs=4 * max(S_U) + 10) as wpool,
            tc.tile_pool(name="asb", bufs=max(F, 1)) as apool,
            tc.tile_pool(name="hmt", bufs=4) as mpool,
            tc.tile_pool(name="pout", bufs=2) as opool,
            tc.tile_pool(name="ps_acc", bufs=3, space="PSUM") as ps_acc,
            tc.tile_pool(name="ps_dec", bufs=4, space="PSUM") as ps_dec,
        ):
            # dma_gather (InstDMAGatherAnt) lives in the gpsimd mlp library;
            # load it before any gpsimd instruction.
            nc.gpsimd.load_library(library_config.mlp)

            # encoder weights split into pieces so tile 0's matmuls start
            # after ~1.3MB (first piece + first x tile) instead of 7.5MB.
            enc_sb = cpool.tile([P, KC * HIDDEN], bf16, tag="encw")
            ENC_SPLIT = [6, 18, 32, KC]
            nc.sync.dma_start(
                out=enc_sb[:, : ENC_SPLIT[0] * HIDDEN],
                in_=encw_d[:, : ENC_SPLIT[0] * HIDDEN],
            )
            si_sb = cpool.tile([P, NBLK * 8], i16, tag="sidx")
            dec_sb = cpool.tile([P, 4 * N_ITEMS], bf16, tag="decw")

            # ---------------- Phase 1: encoder ----------------
            hsbs = []
            for ut in range(UT):
                xb = xpool.tile([P, KC * P], bf16, tag="xb")
                nc.sync.dma_start(out=xb[:], in_=x_d[ut * P : (ut + 1) * P, :])
                if ut == 0:
                    e_lo = ENC_SPLIT[0]
                    for e_hi in ENC_SPLIT[1:]:
                        nc.sync.dma_start(
                            out=enc_sb[:, e_lo * HIDDEN : e_hi * HIDDEN],
                            in_=encw_d[:, e_lo * HIDDEN : e_hi * HIDDEN],
                        )
                        e_lo = e_hi
                h_ps = ps_acc.tile([P, 512], f32, tag="acc")
                for k in range(KC):
                    nc.tensor.matmul(
                        out=h_ps[:, :HIDDEN],
                        lhsT=xb[:, k * P : (k + 1) * P],
                        rhs=enc_sb[:, k * HIDDEN : (k + 1) * HIDDEN],
                        start=(k == 0),
                        stop=(k == KC - 1),
                    )
                # bufs=UT and exactly UT allocations of this tag: every tile
                # keeps its own SBUF-resident buffer for the whole kernel.
                hsb = hpool.tile([P, HPAD], bf16, tag="hsb")
                hsbs.append(hsb)
                nc.scalar.activation(
                    out=hsb[:, :HIDDEN],
                    in_=h_ps[:, :HIDDEN],
                    func=mybir.ActivationFunctionType.Sigmoid,
                )
                nc.vector.memset(hsb[:, HIDDEN:HPAD], 0.0)
                nr = min((ut + 1) * P, UPC) - ut * P  # 68 real rows on tile 19
                nc.sync.dma_start(
                    out=h_loc[ut * P : ut * P + nr, :HIDDEN], in_=hsb[:nr, :HIDDEN]
                )
                if ut == 2:
                    # deferred const loads: issued after the first x tiles so
                    # the encoder pipeline fills before they take bandwidth.
                    nc.sync.dma_start(out=si_sb[:], in_=si_d[:])
                elif ut == 3:
                    nc.sync.dma_start(
                        out=dec_sb[:, : 2 * N_ITEMS], in_=decw_d[:, : 2 * N_ITEMS]
                    )
                elif ut == 5:
                    nc.sync.dma_start(
                        out=dec_sb[:, 2 * N_ITEMS :], in_=decw_d[:, 2 * N_ITEMS :]
                    )
                # ---- Phase 2 (interleaved): chunked all-gather ----
                if (ut + 1) in CC_TILE_BOUNDS:
                    j = CC_TILE_BOUNDS.index(ut + 1)
                    rlo = 0 if j == 0 else CC_ROW_BOUNDS[j - 1]
                    rhi = CC_ROW_BOUNDS[j]
                    nc.gpsimd.collective_compute(
                        "AllGather",
                        mybir.AluOpType.bypass,
                        replica_groups=[list(range(M))],
                        ins=[h_loc[rlo:rhi, :]],
                        outs=[h_full[M * rlo : M * rhi, :]],
                    )

            # ---------------- Phase 3: message passing + decoder ----------------
            # Software-pipelined: message matmuls of tile t are emitted before
            # decoder matmuls of tile t-1, so the PE stays busy while ACT
            # drains agg(t) into hmT(t).
            hmTs = [None] * UT
            asbs = [None] * F
            GTW = max([1] + list(S_U) + list(S_A) + list(S_B))

            ident = cpool.tile([P, P], f32, tag="ident")
            make_identity(nc, ident[:])

            def gather(t, St, boff, lo, hi, q):
                # ONE dma_gather fetches all St*128 source rows: out[p, s, :]
                # = h_full[lo + idx[s*128+p]].  The sliced source AP keeps the
                # dependency on just the all-gather chunks covering [lo, hi).
                gt_all = gpool.tile([P, GTW, HW], bf16, tag="gt")
                nc.gpsimd.dma_gather(
                    gt_all[:, :St, :],
                    h_full[lo:hi, :],
                    si_sb[:, boff * 8 : (boff + St) * 8],
                    St * P,
                    St * P,
                    HW,
                    queue_num=q,
                )
                return gt_all

            def load_wbs(boff, n):
                wbs = []
                for s in range(n):
                    wb = wpool.tile([P, P], bf16, tag="wb")
                    nc.sync.dma_start(out=wb[:], in_=wb_d[boff + s])
                    wbs.append(wb)
                return wbs

            def emit_msg_a(t):
                # primed tile, epoch A: prefix sources + self-loop, parked in
                # SBUF f32.  Runs during the all-gather tail (only needs the
                # first EPC chunks).
                psA = ps_acc.tile([P, 512], f32, tag="acc")
                St = S_A[t]
                gt_all = gather(t, St, BOFF_A[t], 0, PFX, t % 4) if St else None
                wbs = load_wbs(BOFF_A[t], St)
                wbself = wpool.tile([P, P], bf16, tag="wb")
                nc.sync.dma_start(out=wbself[:], in_=wb_d[NBLK + t])
                wbs.append(wbself)
                for c in range(4):
                    for s in range(St + 1):
                        lhsT = (
                            gt_all[:, s, c * 126 : (c + 1) * 126]
                            if s < St
                            else hsbs[t][:, c * 126 : (c + 1) * 126]
                        )
                        nc.tensor.matmul(
                            out=psA[0:126, c * P : (c + 1) * P],
                            lhsT=lhsT,
                            rhs=wbs[s][:],
                            start=(s == 0),
                            stop=(s == St),
                        )
                asb = apool.tile([P, 512], f32, tag="asb")
                asbs[t] = asb
                nc.scalar.activation(
                    out=asb[0:126, :],
                    in_=psA[0:126, :],
                    func=mybir.ActivationFunctionType.Copy,
                )

            def emit_msg_core(t, agg_ps):
                # drain agg psum -> hmT bf16 (+ bias/fill rows via DMA)
                hmT = mpool.tile([P, 512], bf16, tag="hmT")
                hmTs[t] = hmT
                # hidden unit 500 (chunk 3, row 122): decoder-bias unit
                # hidden unit 501 (chunk 3, row 123): row-mask fill unit
                # rows 124-125 are zero padding.  The ACT copy below skips
                # rows 122+ of chunk 3, so this DMA has no dependency on the
                # agg drain and can land during the message matmuls.
                nc.sync.dma_start(
                    out=hmT[122:126, 3 * P : 4 * P],
                    in_=rv_d[0:4, t * P : (t + 1) * P],
                )
                nc.scalar.activation(
                    out=hmT[0:126, 0 : 3 * P],
                    in_=agg_ps[0:126, 0 : 3 * P],
                    func=mybir.ActivationFunctionType.Copy,
                )
                nc.scalar.activation(
                    out=hmT[0:122, 3 * P : 4 * P],
                    in_=agg_ps[0:122, 3 * P : 4 * P],
                    func=mybir.ActivationFunctionType.Copy,
                )

            def emit_msg_b(t):
                # primed tile, epoch B: suffix sources, then the parked
                # epoch-A partial is added back via an exact f32 identity
                # matmul inside the same accumulation group.
                psB = ps_acc.tile([P, 512], f32, tag="acc")
                St = S_B[t]
                gt_all = gather(t, St, BOFF_B[t], PFX, M * UPC, t % 4) if St else None
                wbs = load_wbs(BOFF_B[t], St)
                for c in range(4):
                    for s in range(St):
                        nc.tensor.matmul(
                            out=psB[0:126, c * P : (c + 1) * P],
                            lhsT=gt_all[:, s, c * 126 : (c + 1) * 126],
                            rhs=wbs[s][:],
                            start=(s == 0),
                            stop=False,
                        )
                    nc.tensor.matmul(
                        out=psB[0:126, c * P : (c + 1) * P],
                        lhsT=ident[0:126, 0:126],
                        rhs=asbs[t][0:126, c * P : (c + 1) * P],
                        start=(St == 0),
                        stop=True,
                    )
                emit_msg_core(t, psB)

            def emit_msg(t):
                agg_ps = ps_acc.tile([P, 512], f32, tag="acc")
                St = S_U[t - F]
                boff = BOFF_U[t - F]
                gt_all = gather(t, St, boff, 0, M * UPC, t % 4)
                wbs = load_wbs(boff, St)
                # self-loop block: this core's own h tile straight from SBUF
                # with a diagonal weight block (no gather, no DMA).
                wbself = wpool.tile([P, P], bf16, tag="wb")
                nc.sync.dma_start(out=wbself[:], in_=wb_d[NBLK + t])
                wbs.append(wbself)
                # keep each PSUM sub-region's accumulation group contiguous:
                # interleaved start=True matmuls in one bank clobber each
                # other's accumulation state.
                for c in range(4):
                    for s in range(St + 1):
                        lhsT = (
                            gt_all[:, s, c * 126 : (c + 1) * 126]
                            if s < St
                            else hsbs[t][:, c * 126 : (c + 1) * 126]
                        )
                        nc.tensor.matmul(
                            out=agg_ps[0:126, c * P : (c + 1) * P],
                            lhsT=lhsT,
                            rhs=wbs[s][:],
                            start=(s == 0),
                            stop=(s == St),
                        )
                emit_msg_core(t, agg_ps)

            def emit_dec(t):
                hmT = hmTs[t]
                nu = UPC - t * P if t == UT - 1 else P  # 68 on the last tile
                for half in range(2):
                    # batch 6 x 500-col chunks into one SBUF row-block so the
                    # output DMA moves contiguous 6KB rows.
                    psb = opool.tile([P, 3000], f16, tag="psb")
                    for nn in range(6):
                        n = half * 6 + nn
                        p_ps = ps_dec.tile([P, 512], f32, tag="pps")
                        for c in range(4):
                            nc.tensor.matmul(
                                out=p_ps[:, :500],
                                lhsT=hmT[0:126, c * P : (c + 1) * P],
                                rhs=dec_sb[0:126, c * N_ITEMS + n * 500 : c * N_ITEMS + (n + 1) * 500],
                                start=(c == 0),
                                stop=(c == 3),
                            )
                        nc.vector.tensor_scalar(
                            out=psb[:, nn * 500 : (nn + 1) * 500],
                            in0=p_ps[:, :500],
                            scalar1=R_MAX,
                            scalar2=R_MIN,
                            op0=mybir.AluOpType.min,
                            op1=mybir.AluOpType.max,
                        )
                    nc.sync.dma_start(
                        out=out_d[t * P : t * P + nu, half * 3000 : (half + 1) * 3000],
                        in_=psb[:nu, :],
                    )

            # epoch-A sweep of the primed tiles fills the PE during the
            # all-gather tail; then the usual msg(t) / dec(t-1) interleave.
            for t in range(F):
                emit_msg_a(t)
            for t in range(UT):
                if t < F:
                    emit_msg_b(t)
                else:
                    emit_msg(t)
                if t > 0:
                    emit_dec(t - 1)
            emit_dec(UT - 1)

    nc.finalize()
    return nc


def _prep_host(x, edge_index, edge_weight, ft_n0, ft_n1, fill_const,
               enc_w, enc_b, dec_w, dec_b, conv_w):
    """All host-side preprocessing: sharding, weight prep, edge packing."""
    x = np.asarray(x, np.float32)
    src = np.asarray(edge_index[0], np.int64)
    dst = np.asarray(edge_index[1], np.int64)
    w = np.asarray(edge_weight, np.float32)
    ft_n0 = np.asarray(ft_n0)
    ft_n1 = np.asarray(ft_n1)
    fill = float(np.asarray(fill_const))
    conv = float(np.asarray(conv_w))
    enc_w = np.asarray(enc_w, np.float32)
    enc_b = np.asarray(enc_b, np.float32)
    dec_w = np.asarray(dec_w, np.float32)
    dec_b = np.asarray(dec_b, np.float32)

    rowmask = ft_n0 == 0  # rows forced to fill
    colmask = ft_n1 == 0  # cols forced to fill

    # ---- x per core, transposed to item-major tiles on host ----
    # layout: [UT, 128 (item-in-chunk p), KC, 128 (user u)] so each user
    # tile is one contiguous [128, KC*128] bf16 DMA and lhsT chunks are
    # direct column slices.
    xp = np.zeros((M, UPAD, IPAD), np.float32)
    xp[:, :UPC, :N_ITEMS] = x.reshape(M, UPC, N_ITEMS)
    xp[:, :, N_ITEMS] = 1.0  # encoder-bias input column
    xt_host = np.ascontiguousarray(
        xp.reshape(M, UT, 128, KC, 128).transpose(0, 1, 4, 3, 2)
    ).astype(_bf16).reshape(M, UPAD, KC * 128)

    # ---- encoder weights: [6016, 500] -> [128, 47*500] chunk-major ----
    ewp = np.zeros((IPAD, HIDDEN), np.float32)
    ewp[:N_ITEMS] = enc_w.T
    ewp[N_ITEMS] = enc_b
    enc_host = np.ascontiguousarray(
        ewp.reshape(KC, 128, HIDDEN).transpose(1, 0, 2).reshape(128, KC * HIDDEN)
    ).astype(_bf16)

    # ---- decoder weights with baked column mask / bias / fill units ----
    dw = dec_w.T.copy()  # [500, 6000]
    dw[:, colmask] = 0.0
    hp = np.zeros((HPAD, N_ITEMS), np.float32)
    hp[:HIDDEN] = dw
    hp[HIDDEN] = np.where(colmask, fill, dec_b)  # bias unit
    hp[HIDDEN + 1] = fill  # row-mask fill unit (all cols)
    dec_host = np.zeros((128, 4, N_ITEMS), np.float32)
    dec_host[:126] = hp.reshape(4, 126, N_ITEMS).transpose(1, 0, 2)
    dec_host = np.ascontiguousarray(dec_host.reshape(128, 4 * N_ITEMS)).astype(_bf16)

    # ---- edges: filter masked dst, fold conv_w ----
    keep = ~rowmask[dst]
    src_a = src[keep]
    dst_a = dst[keep]
    w_a = w[keep] * conv

    order = np.argsort(dst_a, kind="stable")
    src_a, dst_a, w_a = src_a[order], dst_a[order], w_a[order]

    core = dst_a // UPC
    ldst = dst_a - core * UPC
    tile_g = core * UT + ldst // 128  # global tile id (sorted ascending)
    din = (ldst % 128).astype(np.int64)

    # gather index into the PADDED all-gathered h table.
    # h_full layout after the uneven chunked all-gather: chunk j covers local
    # rows [lo_j*128, hi_j*128) of every core, concatenated core-major:
    # row = off_j + core * crows_j + (local - lo_j*128)
    src_core = src_a // UPC
    src_loc = src_a % UPC
    bounds_rows = np.array([min(b * 128, UPC) for b in CC_TILE_BOUNDS])
    starts_rows = np.concatenate([[0], bounds_rows[:-1]])
    crows = bounds_rows - starts_rows
    offs = np.concatenate([[0], np.cumsum(M * crows)[:-1]])
    cjs = np.searchsorted(bounds_rows, src_loc, side="right")
    gsrc_e = (
        offs[cjs] + src_core * crows[cjs] + (src_loc - starts_rows[cjs])
    ).astype(np.int64)

    # per-(tile, epoch) block quotas (max over cores, so the SPMD program is
    # identical on every core).  The first F_PRIME dst tiles are split into
    # epoch A (sources in the h_full prefix written by the first EPC
    # all-gather chunks, idx as-is) and epoch B (suffix sources, idx rebased)
    # so epoch A can run during the all-gather tail.
    EPC = 4
    PFX = M * int(bounds_rows[EPC - 1])
    t_of_edge = tile_g % UT
    in_prime = t_of_edge < F_PRIME
    in_sfx = gsrc_e >= PFX

    def pack(sel, rebase, min1_from=None):
        """Pack selected edges into per-tile 128-edge blocks.

        dma_gather index layout: idx j of tile t at column boff[t]*8 + j//16,
        partition j%16, replicated 8x down the 128 partitions.  Padding uses
        index 0 (gathers a real row, multiplied by weight 0).
        """
        tg = tile_g[sel]
        gi_all = gsrc_e[sel] - rebase
        dn = din[sel]
        ww = w_a[sel]
        cnt = np.bincount(tg, minlength=M * UT).reshape(M, UT)
        S_t = np.ceil(cnt.max(axis=0) / 128).astype(np.int64)
        if min1_from is not None:
            S_t[min1_from:] = np.maximum(1, S_t[min1_from:])
        boff = np.concatenate([[0], np.cumsum(S_t)[:-1]])
        nblk = int(S_t.sum())
        si_h = np.zeros((M, 128, nblk * 8), np.int16)
        wb_h = np.zeros((M, nblk, 128, 128), np.float32)
        starts = np.zeros(M * UT + 1, np.int64)
        np.cumsum(cnt.reshape(-1), out=starts[1:])
        for g in range(M * UT):
            c, t = divmod(g, UT)
            St = int(S_t[t])
            if St == 0:
                continue
            n = int(cnt[c, t])
            sl = slice(starts[g], starts[g] + n)
            cap = St * 128
            gi = np.zeros(cap, np.int64)
            wi = np.zeros(cap, np.float32)
            di = np.zeros(cap, np.int64)
            gi[:n] = gi_all[sl]
            wi[:n] = ww[sl]
            di[:n] = dn[sl]
            b0 = int(boff[t])
            wrap = gi.astype(np.int16).reshape(-1, 16).T  # [16, S*8]
            si_h[c, :, b0 * 8 : (b0 + St) * 8] = np.tile(wrap, (8, 1))
            for q in range(St):
                blk = slice(q * 128, (q + 1) * 128)
                wb_h[c, b0 + q][np.arange(128), di[blk]] = wi[blk]
        return S_t, si_h, wb_h

    SA_f, si_A, wb_A = pack(in_prime & ~in_sfx, 0)
    SB_f, si_B, wb_B = pack(in_prime & in_sfx, PFX)
    SU_f, si_U, wb_U = pack(~in_prime, 0, min1_from=F_PRIME)
    S_A = tuple(int(v) for v in SA_f[:F_PRIME])
    S_B = tuple(int(v) for v in SB_f[:F_PRIME])
    S_U = tuple(int(v) for v in SU_f[F_PRIME:])
    si_host = np.concatenate([si_A, si_B, si_U], axis=2)
    wblk_host = np.concatenate([wb_A, wb_B, wb_U], axis=1)
    # diagonal self-loop weight blocks, appended after the gather blocks:
    # block NBLK + t applies (1-conv)*live(d) to the SBUF h tile t.
    lv = np.zeros((M, UPAD), np.float32)
    lv[:, :UPC] = (~rowmask).reshape(M, UPC).astype(np.float32) * (1.0 - conv)
    wself = np.zeros((M, UT, 128, 128), np.float32)
    di128 = np.arange(128)
    for t in range(UT):
        wself[:, t, di128, di128] = lv[:, t * 128 : (t + 1) * 128]
    wblk_host = np.concatenate([wblk_host, wself], axis=1).astype(_bf16)

    # ---- row vectors: bias-unit coeff and row-mask coeff per padded user
    # (rows 2-3 are zero fillers for hmT pad rows 124-125) ----
    rv = np.zeros((M, 4, UPAD), np.float32)
    rm = rowmask.reshape(M, UPC)
    rv[:, 0, :UPC] = (~rm).astype(np.float32)  # bias unit on for live rows
    rv[:, 1, :UPC] = rm.astype(np.float32)     # fill unit on for masked rows
    rv_host = rv.astype(_bf16)

    in_maps = []
    for c in range(M):
        in_maps.append(
            {
                "x": xt_host[c],
                "encw": enc_host,
                "decw": dec_host,
                "sidx": si_host[c],
                "wblk": wblk_host[c],
                "rowvec": rv_host[c],
            }
        )
    return S_A, S_B, S_U, in_maps


def _install_ntff_hook_shim():
    """The agent image's antenv lacks axon_hooks; synthesize it so
    run_bass_kernel_spmd(trace=True) can capture NTFF profiles."""
    import types

    if "antenv.axon_hooks" in sys.modules:
        return
    try:
        from trn_agent_boot.trn_boot import _ntff_profile_via_ctypes
    except ImportError:
        return
    hook = _ntff_profile_via_ctypes("/opt/axon/libaxon_pjrt.so")
    mod = types.ModuleType("antenv.axon_hooks")
    mod._hook = hook
    mod.set_axon_ntff_profile_hook = lambda h: setattr(mod, "_hook", h)
    mod.get_axon_ntff_profile_hook = lambda: mod._hook
    sys.modules["antenv.axon_hooks"] = mod
    try:
        import antenv

        antenv.axon_hooks = mod
    except ImportError:
        pass


LAST_EXEC_NS = None
LAST_RESULTS = None


def kernel(x, edge_index, edge_weight, ft_n0, ft_n1, fill_const,
           enc_w, enc_b, dec_w, dec_b, conv_w):
    global LAST_EXEC_NS, LAST_RESULTS
    from concourse.bass_utils import run_bass_kernel_spmd

    S_A, S_B, S_U, in_maps = _prep_host(
        x, edge_index, edge_weight, ft_n0, ft_n1, fill_const,
        enc_w, enc_b, dec_w, dec_b, conv_w,
    )

    key = (S_A, S_B, S_U)
    if key not in _PROGRAM_CACHE:
        _PROGRAM_CACHE[key] = _build_program(S_A, S_B, S_U)
    nc = _PROGRAM_CACHE[key]

    trace = os.environ.get("KERNEL_TRACE", "0") == "1"
    tmpdir = os.environ.get("KERNEL_TRACE_DIR") or None
    if trace:
        _install_ntff_hook_shim()
    res = run_bass_kernel_spmd(
        nc,
        in_maps,
        core_ids=list(range(M)),
        trace=trace,
        tmpdir=tmpdir,
    )
    LAST_EXEC_NS = res.exec_time_ns
    LAST_RESULTS = res
    out = np.concatenate([res.results[c]["out"] for c in range(M)], axis=0)
    return np.ascontiguousarray(out.astype(np.float32))


# revision 42
# speedup vs baseline: 1.0279x; 1.0279x over previous
"""Trainium2 Bass kernel for nn_Autorec_DG_13116830122688 (AutoRec + GraphConv0D).

Math (reference):
    h   = sigmoid(x @ enc_w.T + enc_b)                      [N, 500]
    agg = segment_sum(h[src] * edge_weight, dst, N)
    hm  = conv_w * agg + (1 - conv_w) * h
    p   = clip(hm @ dec_w.T + dec_b, 1, 5)
    p   = where(ft_n0 == 0 rows, fill, p); where(ft_n1 == 0 cols, fill, p)

Strategy (8 NeuronCores, data-parallel over users):
  - Shard users 2500/core (padded to 2560 = 20x128 tiles).
  - Encoder: x is pre-transposed to item-major [128, KC*128] tiles ON HOST
    (bf16), so each user tile is one contiguous 1.5MB DMA and the 47-chunk
    matmul accumulation runs with no PE transposes.  Encoder bias folded in
    as an extra always-one input column.  ACT sigmoid -> h bf16 (SBUF
    resident for the whole kernel).  Decoder weights and gather indices are
    loaded AFTER the first x tile so the PE starts ~35us earlier.
  - AllGather h (bf16, only the 2500 real rows, 512-wide) in 6 chunks
    overlapped with the encoder so every core can gather any source
    embedding; first chunk after 3 tiles keeps the serial collective stream
    ahead of the encoder, small last chunk keeps the exposed tail short.
  - Message passing: edges are filtered (masked-dst rows dropped), scaled by
    conv_w, self-loops with weight (1-conv_w) added, sorted by dst and packed
    into 128-edge blocks per 128-dst tile.  ONE gpsimd dma_gather per dst
    tile fetches all its source rows (sub-1us issue; int16 indices in the
    16-partition wrapped layout), then each block multiplies a host-built
    [128 edges x 128 dst] sparse weight matrix on the TensorEngine:
    aggT += G.T @ W accumulates in PSUM in hidden-major layout, which feeds
    the decoder with no extra transpose.  The self-loop block reads this
    core's h directly from SBUF (no DMA).
  - Decoder: p = hmT.T @ dec_w.T with the column mask and fill constant baked
    into host-prepped weights, plus two extra hidden units carrying the decoder
    bias and the row-mask fill. Single DVE instruction clips to [1, 5] and
    emits fp16 (upcast to f32 on host).  Decoder of tile t-1 is emitted after
    message matmuls of tile t so the PE never idles waiting on the hmT copy.
"""

import os
import sys

import numpy as np

for _p in ("/opt/trn_rl_repo",):
    if _p not in sys.path and os.path.isdir(_p):
        sys.path.insert(0, _p)

import ml_dtypes  # noqa: E402

# ---- problem constants (hardcoded per contest rules) ----
N_USERS = 20000
N_ITEMS = 6000
HIDDEN = 500
M = 8  # cores
UPC = N_USERS // M  # 2500 users per core
UT = 20  # user tiles per core
UPAD = UT * 128  # 2560
KC = 47  # item chunks of 128 (6016 = 47*128 >= 6001 incl. bias col)
IPAD = KC * 128  # 6016
HPAD = 504  # hidden padded: 4 chunks of 126 (500 real + bias/mask units)
NCH = 12  # decoder output chunks of 500 (12*500 = 6000)
R_MIN, R_MAX = 1.0, 5.0
# all-gather chunk boundaries in user tiles (cumulative); early small first
# chunk starts the serial collective stream ASAP (the init barrier gates the
# first trigger at ~60us anyway), small last chunk keeps the exposed tail
# after the encoder short.
CC_TILE_BOUNDS = [3, 7, 11, 15, 18, 20]
# number of "primed" dst tiles whose epoch-A message pass runs during the
# all-gather tail.  Measured on hardware: the prime's prefix gathers steal
# interconnect bandwidth from the in-flight all-gather chunks and push the
# collective's completion out by as much PE time as they fill, so it nets
# zero at best.  Disabled.
F_PRIME = 0

_bf16 = ml_dtypes.bfloat16

_PROGRAM_CACHE = {}


def _build_program(S_A, S_B, S_U):
    """Build the SPMD Bass program.

    The first F = len(S_A) dst tiles are "primed": their message matmuls are
    split into epoch A (sources in the h_full prefix covered by the first
    EPC all-gather chunks, plus the self-loop) and epoch B (sources in the
    suffix).  Epoch A runs during the all-gather tail, its partial agg is
    parked in SBUF f32 and added back into epoch B's PSUM group via an exact
    f32 identity matmul.  Tiles F..UT-1 use a single unified pass (S_U).
    """
    import concourse.bass as bass
    import concourse.bacc as bacc
    import concourse.mybir as mybir
    from concourse import library_config
    from concourse.masks import make_identity
    from concourse.tile import TileContext

    P = 128
    f32 = mybir.dt.float32
    f16 = mybir.dt.float16
    bf16 = mybir.dt.bfloat16
    F = len(S_A)
    NBLK_A = sum(S_A)
    NBLK_B = sum(S_B)
    NBLK = NBLK_A + NBLK_B + sum(S_U)
    BOFF_A = [sum(S_A[:t]) for t in range(F)]
    BOFF_B = [NBLK_A + sum(S_B[:t]) for t in range(F)]
    BOFF_U = [NBLK_A + NBLK_B + sum(S_U[:t]) for t in range(UT - F)]

    nc = bacc.Bacc(
        "TRN2",
        target_bir_lowering=False,
        debug=False,
        num_devices=M,
        num_swdge_queues=4,
    )

    # x pre-transposed on host: row ut*128+p (item-in-chunk), col k*128+u
    x_d = nc.declare_dram_parameter("x", [UPAD, KC * P], bf16, isOutput=False)
    encw_d = nc.declare_dram_parameter("encw", [P, KC * HIDDEN], bf16, isOutput=False)
    decw_d = nc.declare_dram_parameter("decw", [P, 4 * N_ITEMS], bf16, isOutput=False)
    i16 = mybir.dt.int16
    # gather indices for dma_gather: idx j of tile t at column boff[t]*8 +
    # j//16, partition j%16, replicated 8x down the 128 partitions.
    si_d = nc.declare_dram_parameter("sidx", [P, NBLK * 8], i16, isOutput=False)
    wb_d = nc.declare_dram_parameter("wblk", [NBLK + UT, P, P], bf16, isOutput=False)
    rv_d = nc.declare_dram_parameter("rowvec", [4, UPAD], bf16, isOutput=False)
    out_d = nc.declare_dram_parameter("out", [UPC, N_ITEMS], f16, isOutput=True)

    # only the 2500 real rows travel through the all-gather.  Rows are 512
    # wide (dma_gather needs a 256B-multiple row stride); cols 500-511 are
    # never written — the garbage lands only in hmT rows that the rv DMA
    # overwrites (hidden 500+) or that carry zero decoder weight.
    HW = 512
    h_loc = nc.dram_tensor("h_loc", [UPC, HW], bf16)
    h_full = nc.dram_tensor("h_full", [M * UPC, HW], bf16, addr_space="Shared")
    CC_ROW_BOUNDS = [min(b * 128, UPC) for b in CC_TILE_BOUNDS]
    EPC = 4  # all-gather chunks forming the h_full prefix (epoch A)
    PFX = M * CC_ROW_BOUNDS[EPC - 1] * 1  # prefix rows: chunk-major layout

    with TileContext(nc) as tc:
        with (
            tc.tile_pool(name="const", bufs=1) as cpool,
            tc.tile_pool(name="xin", bufs=2) as xpool,
            tc.tile_pool(name="hsb", bufs=UT) as hpool,
            tc.tile_pool(name="gat", bufs=3 if F > 0 else 4) as gpool,
            tc.tile_pool(name="wbl", bufs=4 * max(S_U) + 10) as wpool,
            tc.tile_pool(name="asb", bufs=max(F, 1)) as apool,
            tc.tile_pool(name="hmt", bufs=4) as mpool,
            tc.tile_pool(name="pout", bufs=2) as opool,
            tc.tile_pool(name="ps_acc", bufs=3, space="PSUM") as ps_acc,
            tc.tile_pool(name="ps_dec", bufs=4, space="PSUM") as ps_dec,
        ):
            # dma_gather (InstDMAGatherAnt) lives in the gpsimd mlp library;
            # load it before any gpsimd instruction.
            nc.gpsimd.load_library(library_config.mlp)

            # encoder weights split into pieces so tile 0's matmuls start
            # after ~1.3MB (first piece + first x tile) instead of 7.5MB.
            enc_sb = cpool.tile([P, KC * HIDDEN], bf16, tag="encw")
            ENC_SPLIT = [6, 18, 32, KC]
            nc.sync.dma_start(
                out=enc_sb[:, : ENC_SPLIT[0] * HIDDEN],
                in_=encw_d[:, : ENC_SPLIT[0] * HIDDEN],
            )
            si_sb = cpool.tile([P, NBLK * 8], i16, tag="sidx")
            dec_sb = cpool.tile([P, 4 * N_ITEMS], bf16, tag="decw")

            # ---------------- Phase 1: encoder ----------------
            hsbs = []
            for ut in range(UT):
                xb = xpool.tile([P, KC * P], bf16, tag="xb")
                nc.sync.dma_start(out=xb[:], in_=x_d[ut * P : (ut + 1) * P, :])
                if ut == 0:
                    e_lo = ENC_SPLIT[0]
                    for e_hi in ENC_SPLIT[1:]:
                        nc.sync.dma_start(
                            out=enc_sb[:, e_lo * HIDDEN : e_hi * HIDDEN],
                            in_=encw_d[:, e_lo * HIDDEN : e_hi * HIDDEN],
                        )
                        e_lo = e_hi
                h_ps = ps_acc.tile([P, 512], f32, tag="acc")
                for k in range(KC):
                    nc.tensor.matmul(
                        out=h_ps[:, :HIDDEN],
                        lhsT=xb[:, k * P : (k + 1) * P],
                        rhs=enc_sb[:, k * HIDDEN : (k + 1) * HIDDEN],
                        start=(k == 0),
                        stop=(k == KC - 1),
                    )
                # bufs=UT and exactly UT allocations of this tag: every tile
                # keeps its own SBUF-resident buffer for the whole kernel.
                hsb = hpool.tile([P, HPAD], bf16, tag="hsb")
                hsbs.append(hsb)
                nc.scalar.activation(
                    out=hsb[:, :HIDDEN],
                    in_=h_ps[:, :HIDDEN],
                    func=mybir.ActivationFunctionType.Sigmoid,
                )
                nc.vector.memset(hsb[:, HIDDEN:HPAD], 0.0)
                nr = min((ut + 1) * P, UPC) - ut * P  # 68 real rows on tile 19
                nc.sync.dma_start(
                    out=h_loc[ut * P : ut * P + nr, :HIDDEN], in_=hsb[:nr, :HIDDEN]
                )
                if ut == 2:
                    # deferred const loads: issued after the first x tiles so
                    # the encoder pipeline fills before they take bandwidth.
                    nc.sync.dma_start(out=si_sb[:], in_=si_d[:])
                elif ut == 3:
                    nc.sync.dma_start(
                        out=dec_sb[:, : 2 * N_ITEMS], in_=decw_d[:, : 2 * N_ITEMS]
                    )
                elif ut == 5:
                    nc.sync.dma_start(
                        out=dec_sb[:, 2 * N_ITEMS :], in_=decw_d[:, 2 * N_ITEMS :]
                    )
                # ---- Phase 2 (interleaved): chunked all-gather ----
                if (ut + 1) in CC_TILE_BOUNDS:
                    j = CC_TILE_BOUNDS.index(ut + 1)
                    rlo = 0 if j == 0 else CC_ROW_BOUNDS[j - 1]
                    rhi = CC_ROW_BOUNDS[j]
                    nc.gpsimd.collective_compute(
                        "AllGather",
                        mybir.AluOpType.bypass,
                        replica_groups=[list(range(M))],
                        ins=[h_loc[rlo:rhi, :]],
                        outs=[h_full[M * rlo : M * rhi, :]],
                    )

            # ---------------- Phase 3: message passing + decoder ----------------
            # Software-pipelined: message matmuls of tile t are emitted before
            # decoder matmuls of tile t-1, so the PE stays busy while ACT
            # drains agg(t) into hmT(t).
            hmTs = [None] * UT
            asbs = [None] * F
            GTW = max([1] + list(S_U) + list(S_A) + list(S_B))

            ident = cpool.tile([P, P], f32, tag="ident")
            make_identity(nc, ident[:])

            def gather(t, St, boff, lo, hi, q):
                # ONE dma_gather fetches all St*128 source rows: out[p, s, :]
                # = h_full[lo + idx[s*128+p]].  The sliced source AP keeps the
                # dependency on just the all-gather chunks covering [lo, hi).
                gt_all = gpool.tile([P, GTW, HW], bf16, tag="gt")
                nc.gpsimd.dma_gather(
                    gt_all[:, :St, :],
                    h_full[lo:hi, :],
                    si_sb[:, boff * 8 : (boff + St) * 8],
                    St * P,
                    St * P,
                    HW,
                    queue_num=q,
                )
                return gt_all

            def load_wbs(boff, n):
                wbs = []
                for s in range(n):
                    wb = wpool.tile([P, P], bf16, tag="wb")
                    nc.sync.dma_start(out=wb[:], in_=wb_d[boff + s])
                    wbs.append(wb)
                return wbs

            def emit_msg_a(t):
                # primed tile, epoch A: prefix sources + self-loop, parked in
                # SBUF f32.  Runs during the all-gather tail (only needs the
                # first EPC chunks).
                psA = ps_acc.tile([P, 512], f32, tag="acc")
                St = S_A[t]
                gt_all = gather(t, St, BOFF_A[t], 0, PFX, t % 4) if St else None
                wbs = load_wbs(BOFF_A[t], St)
                wbself = wpool.tile([P, P], bf16, tag="wb")
                nc.sync.dma_start(out=wbself[:], in_=wb_d[NBLK + t])
                wbs.append(wbself)
                for c in range(4):
                    for s in range(St + 1):
                        lhsT = (
                            gt_all[:, s, c * 126 : (c + 1) * 126]
                            if s < St
                            else hsbs[t][:, c * 126 : (c + 1) * 126]
                        )
                        nc.tensor.matmul(
                            out=psA[0:126, c * P : (c + 1) * P],
                            lhsT=lhsT,
                            rhs=wbs[s][:],
                            start=(s == 0),
                            stop=(s == St),
                        )
                asb = apool.tile([P, 512], f32, tag="asb")
                asbs[t] = asb
                nc.scalar.activation(
                    out=asb[0:126, :],
                    in_=psA[0:126, :],
                    func=mybir.ActivationFunctionType.Copy,
                )

            def emit_msg_core(t, agg_ps):
                # drain agg psum -> hmT bf16 (+ bias/fill rows via DMA)
                hmT = mpool.tile([P, 512], bf16, tag="hmT")
                hmTs[t] = hmT
                # hidden unit 500 (chunk 3, row 122): decoder-bias unit
                # hidden unit 501 (chunk 3, row 123): row-mask fill unit
                # rows 124-125 are zero padding.  The ACT copy below skips
                # rows 122+ of chunk 3, so this DMA has no dependency on the
                # agg drain and can land during the message matmuls.
                nc.sync.dma_start(
                    out=hmT[122:126, 3 * P : 4 * P],
                    in_=rv_d[0:4, t * P : (t + 1) * P],
                )
                nc.scalar.activation(
                    out=hmT[0:126, 0 : 3 * P],
                    in_=agg_ps[0:126, 0 : 3 * P],
                    func=mybir.ActivationFunctionType.Copy,
                )
                nc.scalar.activation(
                    out=hmT[0:122, 3 * P : 4 * P],
                    in_=agg_ps[0:122, 3 * P : 4 * P],
                    func=mybir.ActivationFunctionType.Copy,
                )

            def emit_msg_b(t):
                # primed tile, epoch B: suffix sources, then the parked
                # epoch-A partial is added back via an exact f32 identity
                # matmul inside the same accumulation group.
                psB = ps_acc.tile([P, 512], f32, tag="acc")
                St = S_B[t]
                gt_all = gather(t, St, BOFF_B[t], PFX, M * UPC, t % 4) if St else None
                wbs = load_wbs(BOFF_B[t], St)
                for c in range(4):
                    for s in range(St):
                        nc.tensor.matmul(
                            out=psB[0:126, c * P : (c + 1) * P],
                            lhsT=gt_all[:, s, c * 126 : (c + 1) * 126],
                            rhs=wbs[s][:],
                            start=(s == 0),
                            stop=False,
                        )
                    nc.tensor.matmul(
                        out=psB[0:126, c * P : (c + 1) * P],
                        lhsT=ident[0:126, 0:126],
                        rhs=asbs[t][0:126, c * P : (c + 1) * P],
                        start=(St == 0),
                        stop=True,
                    )
                emit_msg_core(t, psB)

            def emit_msg(t):
                agg_ps = ps_acc.tile([P, 512], f32, tag="acc")
                St = S_U[t - F]
                boff = BOFF_U[t - F]
                gt_all = gather(t, St, boff, 0, M * UPC, t % 4)
                wbs = load_wbs(boff, St)
                # self-loop block: this core's own h tile straight from SBUF
                # with a diagonal weight block (no gather, no DMA).
                wbself = wpool.tile([P, P], bf16, tag="wb")
                nc.sync.dma_start(out=wbself[:], in_=wb_d[NBLK + t])
                wbs.append(wbself)
                # keep each PSUM sub-region's accumulation group contiguous:
                # interleaved start=True matmuls in one bank clobber each
                # other's accumulation state.
                for c in range(4):
                    for s in range(St + 1):
                        lhsT = (
                            gt_all[:, s, c * 126 : (c + 1) * 126]
                            if s < St
                            else hsbs[t][:, c * 126 : (c + 1) * 126]
                        )
                        nc.tensor.matmul(
                            out=agg_ps[0:126, c * P : (c + 1) * P],
                            lhsT=lhsT,
                            rhs=wbs[s][:],
                            start=(s == 0),
                            stop=(s == St),
                        )
                emit_msg_core(t, agg_ps)

            def emit_dec(t):
                hmT = hmTs[t]
                nu = UPC - t * P if t == UT - 1 else P  # 68 on the last tile
                for half in range(2):
                    # batch 6 x 500-col chunks into one SBUF row-block so the
                    # output DMA moves contiguous 6KB rows.
                    psb = opool.tile([P, 3000], f16, tag="psb")
                    for nn in range(6):
                        n = half * 6 + nn
                        p_ps = ps_dec.tile([P, 512], f32, tag="pps")
                        for c in range(4):
                            nc.tensor.matmul(
                                out=p_ps[:, :500],
                                lhsT=hmT[0:126, c * P : (c + 1) * P],
                                rhs=dec_sb[0:126, c * N_ITEMS + n * 500 : c * N_ITEMS + (n + 1) * 500],
                                start=(c == 0),
                                stop=(c == 3),
                            )
                        nc.vector.tensor_scalar(
                            out=psb[:, nn * 500 : (nn + 1) * 500],
                            in0=p_ps[:, :500],
                            scalar1=R_MAX,
                            scalar2=R_MIN,
                            op0=mybir.AluOpType.min,
                            op1=mybir.AluOpType.max,
                        )
                    nc.sync.dma_start(
                        out=out_d[t * P : t * P + nu, half * 3000 : (half + 1) * 3000],
                        in_=psb[:nu, :],
                    )

            # epoch-A sweep of the primed tiles fills the PE during the
            # all-gather tail; then the usual msg(t) / dec(t-1) interleave.
            for t in range(F):
                emit_msg_a(t)
            for t in range(UT):
                if t < F:
                    emit_msg_b(t)
                else:
                    emit_msg(t)
                if t > 0:
                    emit_dec(t - 1)
            emit_dec(UT - 1)

    nc.finalize()
    return nc


def _prep_host(x, edge_index, edge_weight, ft_n0, ft_n1, fill_const,
               enc_w, enc_b, dec_w, dec_b, conv_w):
    """All host-side preprocessing: sharding, weight prep, edge packing."""
    x = np.asarray(x, np.float32)
    src = np.asarray(edge_index[0], np.int64)
    dst = np.asarray(edge_index[1], np.int64)
    w = np.asarray(edge_weight, np.float32)
    ft_n0 = np.asarray(ft_n0)
    ft_n1 = np.asarray(ft_n1)
    fill = float(np.asarray(fill_const))
    conv = float(np.asarray(conv_w))
    enc_w = np.asarray(enc_w, np.float32)
    enc_b = np.asarray(enc_b, np.float32)
    dec_w = np.asarray(dec_w, np.float32)
    dec_b = np.asarray(dec_b, np.float32)

    rowmask = ft_n0 == 0  # rows forced to fill
    colmask = ft_n1 == 0  # cols forced to fill

    # ---- x per core, transposed to item-major tiles on host ----
    # layout: [UT, 128 (item-in-chunk p), KC, 128 (user u)] so each user
    # tile is one contiguous [128, KC*128] bf16 DMA and lhsT chunks are
    # direct column slices.
    xp = np.zeros((M, UPAD, IPAD), np.float32)
    xp[:, :UPC, :N_ITEMS] = x.reshape(M, UPC, N_ITEMS)
    xp[:, :, N_ITEMS] = 1.0  # encoder-bias input column
    xt_host = np.ascontiguousarray(
        xp.reshape(M, UT, 128, KC, 128).transpose(0, 1, 4, 3, 2)
    ).astype(_bf16).reshape(M, UPAD, KC * 128)

    # ---- encoder weights: [6016, 500] -> [128, 47*500] chunk-major ----
    ewp = np.zeros((IPAD, HIDDEN), np.float32)
    ewp[:N_ITEMS] = enc_w.T
    ewp[N_ITEMS] = enc_b
    enc_host = np.ascontiguousarray(
        ewp.reshape(KC, 128, HIDDEN).transpose(1, 0, 2).reshape(128, KC * HIDDEN)
    ).astype(_bf16)

    # ---- decoder weights with baked column mask / bias / fill units ----
    dw = dec_w.T.copy()  # [500, 6000]
    dw[:, colmask] = 0.0
    hp = np.zeros((HPAD, N_ITEMS), np.float32)
    hp[:HIDDEN] = dw
    hp[HIDDEN] = np.where(colmask, fill, dec_b)  # bias unit
    hp[HIDDEN + 1] = fill  # row-mask fill unit (all cols)
    dec_host = np.zeros((128, 4, N_ITEMS), np.float32)
    dec_host[:126] = hp.reshape(4, 126, N_ITEMS).transpose(1, 0, 2)
    dec_host = np.ascontiguousarray(dec_host.reshape(128, 4 * N_ITEMS)).astype(_bf16)

    # ---- edges: filter masked dst, fold conv_w ----
    keep = ~rowmask[dst]
    src_a = src[keep]
    dst_a = dst[keep]
    w_a = w[keep] * conv

    order = np.argsort(dst_a, kind="stable")
    src_a, dst_a, w_a = src_a[order], dst_a[order], w_a[order]

    core = dst_a // UPC
    ldst = dst_a - core * UPC
    tile_g = core * UT + ldst // 128  # global tile id (sorted ascending)
    din = (ldst % 128).astype(np.int64)

    # gather index into the PADDED all-gathered h table.
    # h_full layout after the uneven chunked all-gather: chunk j covers local
    # rows [lo_j*128, hi_j*128) of every core, concatenated core-major:
    # row = off_j + core * crows_j + (local - lo_j*128)
    src_core = src_a // UPC
    src_loc = src_a % UPC
    bounds_rows = np.array([min(b * 128, UPC) for b in CC_TILE_BOUNDS])
    starts_rows = np.concatenate([[0], bounds_rows[:-1]])
    crows = bounds_rows - starts_rows
    offs = np.concatenate([[0], np.cumsum(M * crows)[:-1]])
    cjs = np.searchsorted(bounds_rows, src_loc, side="right")
    gsrc_e = (
        offs[cjs] + src_core * crows[cjs] + (src_loc - starts_rows[cjs])
    ).astype(np.int64)

    # per-(tile, epoch) block quotas (max over cores, so the SPMD program is
    # identical on every core).  The first F_PRIME dst tiles are split into
    # epoch A (sources in the h_full prefix written by the first EPC
    # all-gather chunks, idx as-is) and epoch B (suffix sources, idx rebased)
    # so epoch A can run during the all-gather tail.
    EPC = 4
    PFX = M * int(bounds_rows[EPC - 1])
    t_of_edge = tile_g % UT
    in_prime = t_of_edge < F_PRIME
    in_sfx = gsrc_e >= PFX

    def pack(sel, rebase, min1_from=None):
        """Pack selected edges into per-tile 128-edge blocks.

        dma_gather index layout: idx j of tile t at column boff[t]*8 + j//16,
        partition j%16, replicated 8x down the 128 partitions.  Padding uses
        index 0 (gathers a real row, multiplied by weight 0).
        """
        tg = tile_g[sel]
        gi_all = gsrc_e[sel] - rebase
        dn = din[sel]
        ww = w_a[sel]
        cnt = np.bincount(tg, minlength=M * UT).reshape(M, UT)
        S_t = np.ceil(cnt.max(axis=0) / 128).astype(np.int64)
        if min1_from is not None:
            S_t[min1_from:] = np.maximum(1, S_t[min1_from:])
        boff = np.concatenate([[0], np.cumsum(S_t)[:-1]])
        nblk = int(S_t.sum())
        si_h = np.zeros((M, 128, nblk * 8), np.int16)
        wb_h = np.zeros((M, nblk, 128, 128), np.float32)
        starts = np.zeros(M * UT + 1, np.int64)
        np.cumsum(cnt.reshape(-1), out=starts[1:])
        for g in range(M * UT):
            c, t = divmod(g, UT)
            St = int(S_t[t])
            if St == 0:
                continue
            n = int(cnt[c, t])
            sl = slice(starts[g], starts[g] + n)
            cap = St * 128
            gi = np.zeros(cap, np.int64)
            wi = np.zeros(cap, np.float32)
            di = np.zeros(cap, np.int64)
            gi[:n] = gi_all[sl]
            wi[:n] = ww[sl]
            di[:n] = dn[sl]
            b0 = int(boff[t])
            wrap = gi.astype(np.int16).reshape(-1, 16).T  # [16, S*8]
            si_h[c, :, b0 * 8 : (b0 + St) * 8] = np.tile(wrap, (8, 1))
            for q in range(St):
                blk = slice(q * 128, (q + 1) * 128)
                wb_h[c, b0 + q][np.arange(128), di[blk]] = wi[blk]
        return S_t, si_h, wb_h

    SA_f, si_A, wb_A = pack(in_prime & ~in_sfx, 0)
    SB_f, si_B, wb_B = pack(in_prime & in_sfx, PFX)
    SU_f, si_U, wb_U = pack(~in_prime, 0, min1_from=F_PRIME)
    S_A = tuple(int(v) for v in SA_f[:F_PRIME])
    S_B = tuple(int(v) for v in SB_f[:F_PRIME])
    S_U = tuple(int(v) for v in SU_f[F_PRIME:])
    si_host = np.concatenate([si_A, si_B, si_U], axis=2)
    wblk_host = np.concatenate([wb_A, wb_B, wb_U], axis=1)
    # diagonal self-loop weight blocks, appended after the gather blocks:
    # block NBLK + t applies (1-conv)*live(d) to the SBUF h tile t.
    lv = np.zeros((M, UPAD), np.float32)
    lv[:, :UPC] = (~rowmask).reshape(M, UPC).astype(np.float32) * (1.0 - conv)
    wself = np.zeros((M, UT, 128, 128), np.float32)
    di128 = np.arange(128)
    for t in range(UT):
        wself[:, t, di128, di128] = lv[:, t * 128 : (t + 1) * 128]
    wblk_host = np.concatenate([wblk_host, wself], axis=1).astype(_bf16)

    # ---- row vectors: bias-unit coeff and row-mask coeff per padded user
    # (rows 2-3 are zero fillers for hmT pad rows 124-125) ----
    rv = np.zeros((M, 4, UPAD), np.float32)
    rm = rowmask.reshape(M, UPC)
    rv[:, 0, :UPC] = (~rm).astype(np.float32)  # bias unit on for live rows
    rv[:, 1, :UPC] = rm.astype(np.float32)     # fill unit on for masked rows
    rv_host = rv.astype(_bf16)

    in_maps = []
    for c in range(M):
        in_maps.append(
            {
                "x": xt_host[c],
                "encw": enc_host,
                "decw": dec_host,
                "sidx": si_host[c],
                "wblk": wblk_host[c],
                "rowvec": rv_host[c],
            }
        )
    return S_A, S_B, S_U, in_maps


def _install_ntff_hook_shim():
    """The agent image's antenv lacks axon_hooks; synthesize it so
    run_bass_kernel_spmd(trace=True) can capture NTFF profiles."""
    import types

    if "antenv.axon_hooks" in sys.modules:
        return
    try:
        from trn_agent_boot.trn_boot import _ntff_profile_via_ctypes
    except ImportError:
        return
    hook = _ntff_profile_via_ctypes("/opt/axon/libaxon_pjrt.so")
    mod = types.ModuleType("antenv.axon_hooks")
    mod._hook = hook
    mod.set_axon_ntff_profile_hook = lambda h: setattr(mod, "_hook", h)
    mod.get_axon_ntff_profile_hook = lambda: mod._hook
    sys.modules["antenv.axon_hooks"] = mod
    try:
        import antenv

        antenv.axon_hooks = mod
    except ImportError:
        pass


LAST_EXEC_NS = None
LAST_RESULTS = None


def kernel(x, edge_index, edge_weight, ft_n0, ft_n1, fill_const,
           enc_w, enc_b, dec_w, dec_b, conv_w):
    global LAST_EXEC_NS, LAST_RESULTS
    from concourse.bass_utils import run_bass_kernel_spmd

    S_A, S_B, S_U, in_maps = _prep_host(
        x, edge_index, edge_weight, ft_n0, ft_n1, fill_const,
        enc_w, enc_b, dec_w, dec_b, conv_w,
    )

    key = (S_A, S_B, S_U)
    if key not in _PROGRAM_CACHE:
        _PROGRAM_CACHE[key] = _build_program(S_A, S_B, S_U)
    nc = _PROGRAM_CACHE[key]

    trace = os.environ.get("KERNEL_TRACE", "0") == "1"
    tmpdir = os.environ.get("KERNEL_TRACE_DIR") or None
    if trace:
        _install_ntff_hook_shim()
    res = run_bass_kernel_spmd(
        nc,
        in_maps,
        core_ids=list(range(M)),
        trace=trace,
        tmpdir=tmpdir,
    )
    LAST_EXEC_NS = res.exec_time_ns
    LAST_RESULTS = res
    out = np.concatenate([res.results[c]["out"] for c in range(M)], axis=0)
    return np.ascontiguousarray(out.astype(np.float32))


# revision 44
# speedup vs baseline: 1.0287x; 1.0008x over previous
"""Trainium2 Bass kernel for nn_Autorec_DG_13116830122688 (AutoRec + GraphConv0D).

Math (reference):
    h   = sigmoid(x @ enc_w.T + enc_b)                      [N, 500]
    agg = segment_sum(h[src] * edge_weight, dst, N)
    hm  = conv_w * agg + (1 - conv_w) * h
    p   = clip(hm @ dec_w.T + dec_b, 1, 5)
    p   = where(ft_n0 == 0 rows, fill, p); where(ft_n1 == 0 cols, fill, p)

Strategy (8 NeuronCores, data-parallel over users):
  - Shard users 2500/core (padded to 2560 = 20x128 tiles).
  - Encoder: x is pre-transposed to item-major [128, KC*128] tiles ON HOST
    (bf16), so each user tile is one contiguous 1.5MB DMA and the 47-chunk
    matmul accumulation runs with no PE transposes.  Encoder bias folded in
    as an extra always-one input column.  ACT sigmoid -> h bf16 (SBUF
    resident for the whole kernel).  Decoder weights and gather indices are
    loaded AFTER the first x tile so the PE starts ~35us earlier.
  - AllGather h (bf16, only the 2500 real rows, 512-wide) in 6 chunks
    overlapped with the encoder so every core can gather any source
    embedding; first chunk after 3 tiles keeps the serial collective stream
    ahead of the encoder, small last chunk keeps the exposed tail short.
  - Message passing: edges are filtered (masked-dst rows dropped), scaled by
    conv_w, self-loops with weight (1-conv_w) added, sorted by dst and packed
    into 128-edge blocks per 128-dst tile.  ONE gpsimd dma_gather per dst
    tile fetches all its source rows (sub-1us issue; int16 indices in the
    16-partition wrapped layout), then each block multiplies a host-built
    [128 edges x 128 dst] sparse weight matrix on the TensorEngine:
    aggT += G.T @ W accumulates in PSUM in hidden-major layout, which feeds
    the decoder with no extra transpose.  The self-loop block reads this
    core's h directly from SBUF (no DMA).
  - Decoder: p = hmT.T @ dec_w.T with the column mask and fill constant baked
    into host-prepped weights, plus two extra hidden units carrying the decoder
    bias and the row-mask fill. Single DVE instruction clips to [1, 5] and
    emits fp16 (upcast to f32 on host).  Decoder of tile t-1 is emitted after
    message matmuls of tile t so the PE never idles waiting on the hmT copy.
"""

import os
import sys

import numpy as np

for _p in ("/opt/trn_rl_repo",):
    if _p not in sys.path and os.path.isdir(_p):
        sys.path.insert(0, _p)

import ml_dtypes  # noqa: E402

# ---- problem constants (hardcoded per contest rules) ----
N_USERS = 20000
N_ITEMS = 6000
HIDDEN = 500
M = 8  # cores
UPC = N_USERS // M  # 2500 users per core
UT = 20  # user tiles per core
UPAD = UT * 128  # 2560
KC = 47  # item chunks of 128 (6016 = 47*128 >= 6001 incl. bias col)
IPAD = KC * 128  # 6016
HPAD = 504  # hidden padded: 4 chunks of 126 (500 real + bias/mask units)
NCH = 12  # decoder output chunks of 500 (12*500 = 6000)
R_MIN, R_MAX = 1.0, 5.0
# all-gather chunk boundaries in user tiles (cumulative); early small first
# chunk starts the serial collective stream ASAP (the init barrier gates the
# first trigger at ~60us anyway), small last chunk keeps the exposed tail
# after the encoder short.
CC_TILE_BOUNDS = [3, 7, 11, 15, 18, 20]
# number of "primed" dst tiles whose epoch-A message pass runs during the
# all-gather tail.  Measured on hardware: the prime's prefix gathers steal
# interconnect bandwidth from the in-flight all-gather chunks and push the
# collective's completion out by as much PE time as they fill, so it nets
# zero at best.  Disabled.
F_PRIME = 0

_bf16 = ml_dtypes.bfloat16

_PROGRAM_CACHE = {}


def _build_program(S_A, S_B, S_U):
    """Build the SPMD Bass program.

    The first F = len(S_A) dst tiles are "primed": their message matmuls are
    split into epoch A (sources in the h_full prefix covered by the first
    EPC all-gather chunks, plus the self-loop) and epoch B (sources in the
    suffix).  Epoch A runs during the all-gather tail, its partial agg is
    parked in SBUF f32 and added back into epoch B's PSUM group via an exact
    f32 identity matmul.  Tiles F..UT-1 use a single unified pass (S_U).
    """
    import concourse.bass as bass
    import concourse.bacc as bacc
    import concourse.mybir as mybir
    from concourse import library_config
    from concourse.masks import make_identity
    from concourse.tile import TileContext

    P = 128
    f32 = mybir.dt.float32
    f16 = mybir.dt.float16
    bf16 = mybir.dt.bfloat16
    F = len(S_A)
    NBLK_A = sum(S_A)
    NBLK_B = sum(S_B)
    NBLK = NBLK_A + NBLK_B + sum(S_U)
    BOFF_A = [sum(S_A[:t]) for t in range(F)]
    BOFF_B = [NBLK_A + sum(S_B[:t]) for t in range(F)]
    BOFF_U = [NBLK_A + NBLK_B + sum(S_U[:t]) for t in range(UT - F)]

    nc = bacc.Bacc(
        "TRN2",
        target_bir_lowering=False,
        debug=False,
        num_devices=M,
        num_swdge_queues=4,
    )

    # x pre-transposed on host: row ut*128+p (item-in-chunk), col k*128+u
    x_d = nc.declare_dram_parameter("x", [UPAD, KC * P], bf16, isOutput=False)
    encw_d = nc.declare_dram_parameter("encw", [P, KC * HIDDEN], bf16, isOutput=False)
    decw_d = nc.declare_dram_parameter("decw", [P, 4 * N_ITEMS], bf16, isOutput=False)
    i16 = mybir.dt.int16
    # gather indices for dma_gather: idx j of tile t at column boff[t]*8 +
    # j//16, partition j%16, replicated 8x down the 128 partitions.
    si_d = nc.declare_dram_parameter("sidx", [P, NBLK * 8], i16, isOutput=False)
    wb_d = nc.declare_dram_parameter("wblk", [NBLK + UT, P, P], bf16, isOutput=False)
    rv_d = nc.declare_dram_parameter("rowvec", [4, UPAD], bf16, isOutput=False)
    out_d = nc.declare_dram_parameter("out", [UPC, N_ITEMS], f16, isOutput=True)

    # only the 2500 real rows travel through the all-gather.  Rows are 512
    # wide (dma_gather needs a 256B-multiple row stride); cols 500-511 are
    # never written — the garbage lands only in hmT rows that the rv DMA
    # overwrites (hidden 500+) or that carry zero decoder weight.
    HW = 512
    h_loc = nc.dram_tensor("h_loc", [UPC, HW], bf16)
    h_full = nc.dram_tensor("h_full", [M * UPC, HW], bf16, addr_space="Shared")
    CC_ROW_BOUNDS = [min(b * 128, UPC) for b in CC_TILE_BOUNDS]
    EPC = 4  # all-gather chunks forming the h_full prefix (epoch A)
    PFX = M * CC_ROW_BOUNDS[EPC - 1] * 1  # prefix rows: chunk-major layout

    with TileContext(nc) as tc:
        with (
            tc.tile_pool(name="const", bufs=1) as cpool,
            tc.tile_pool(name="xin", bufs=2) as xpool,
            tc.tile_pool(name="hsb", bufs=UT) as hpool,
            tc.tile_pool(name="gat", bufs=3 if F > 0 else 4) as gpool,
            tc.tile_pool(name="wbl", bufs=4 * max(S_U) + 10) as wpool,
            tc.tile_pool(name="asb", bufs=max(F, 1)) as apool,
            tc.tile_pool(name="hmt", bufs=4) as mpool,
            tc.tile_pool(name="pout", bufs=2) as opool,
            tc.tile_pool(name="ps_acc", bufs=3, space="PSUM") as ps_acc,
            tc.tile_pool(name="ps_dec", bufs=5, space="PSUM") as ps_dec,
        ):
            # dma_gather (InstDMAGatherAnt) lives in the gpsimd mlp library;
            # load it before any gpsimd instruction.
            nc.gpsimd.load_library(library_config.mlp)

            # encoder weights split into pieces so tile 0's matmuls start
            # after ~1.3MB (first piece + first x tile) instead of 7.5MB.
            enc_sb = cpool.tile([P, KC * HIDDEN], bf16, tag="encw")
            ENC_SPLIT = [6, 18, 32, KC]
            nc.sync.dma_start(
                out=enc_sb[:, : ENC_SPLIT[0] * HIDDEN],
                in_=encw_d[:, : ENC_SPLIT[0] * HIDDEN],
            )
            si_sb = cpool.tile([P, NBLK * 8], i16, tag="sidx")
            dec_sb = cpool.tile([P, 4 * N_ITEMS], bf16, tag="decw")

            # ---------------- Phase 1: encoder ----------------
            def load_x(ut):
                xb = xpool.tile([P, KC * P], bf16, tag="xb", name=f"xb{ut}")
                nc.sync.dma_start(out=xb[:], in_=x_d[ut * P : (ut + 1) * P, :])
                return xb

            hsbs = []
            xb_next = load_x(0)
            for ut in range(UT):
                xb = xb_next
                if ut == 0:
                    # interleave the remaining encoder-weight pieces with the
                    # x1 load so neither starves the tile-0/1 matmul chain.
                    nc.sync.dma_start(
                        out=enc_sb[:, ENC_SPLIT[0] * HIDDEN : ENC_SPLIT[1] * HIDDEN],
                        in_=encw_d[:, ENC_SPLIT[0] * HIDDEN : ENC_SPLIT[1] * HIDDEN],
                    )
                    xb_next = load_x(1)
                    e_lo = ENC_SPLIT[1]
                    for e_hi in ENC_SPLIT[2:]:
                        nc.sync.dma_start(
                            out=enc_sb[:, e_lo * HIDDEN : e_hi * HIDDEN],
                            in_=encw_d[:, e_lo * HIDDEN : e_hi * HIDDEN],
                        )
                        e_lo = e_hi
                elif ut + 1 < UT:
                    xb_next = load_x(ut + 1)
                h_ps = ps_acc.tile([P, 512], f32, tag="acc")
                for k in range(KC):
                    nc.tensor.matmul(
                        out=h_ps[:, :HIDDEN],
                        lhsT=xb[:, k * P : (k + 1) * P],
                        rhs=enc_sb[:, k * HIDDEN : (k + 1) * HIDDEN],
                        start=(k == 0),
                        stop=(k == KC - 1),
                    )
                # bufs=UT and exactly UT allocations of this tag: every tile
                # keeps its own SBUF-resident buffer for the whole kernel.
                hsb = hpool.tile([P, HPAD], bf16, tag="hsb")
                hsbs.append(hsb)
                nc.scalar.activation(
                    out=hsb[:, :HIDDEN],
                    in_=h_ps[:, :HIDDEN],
                    func=mybir.ActivationFunctionType.Sigmoid,
                )
                nc.vector.memset(hsb[:, HIDDEN:HPAD], 0.0)
                nr = min((ut + 1) * P, UPC) - ut * P  # 68 real rows on tile 19
                nc.sync.dma_start(
                    out=h_loc[ut * P : ut * P + nr, :HIDDEN], in_=hsb[:nr, :HIDDEN]
                )
                if ut == 2:
                    # deferred const loads: issued after the first x tiles so
                    # the encoder pipeline fills before they take bandwidth.
                    nc.sync.dma_start(out=si_sb[:], in_=si_d[:])
                elif ut == 3:
                    nc.sync.dma_start(
                        out=dec_sb[:, : 2 * N_ITEMS], in_=decw_d[:, : 2 * N_ITEMS]
                    )
                elif ut == 5:
                    nc.sync.dma_start(
                        out=dec_sb[:, 2 * N_ITEMS :], in_=decw_d[:, 2 * N_ITEMS :]
                    )
                # ---- Phase 2 (interleaved): chunked all-gather ----
                if (ut + 1) in CC_TILE_BOUNDS:
                    j = CC_TILE_BOUNDS.index(ut + 1)
                    rlo = 0 if j == 0 else CC_ROW_BOUNDS[j - 1]
                    rhi = CC_ROW_BOUNDS[j]
                    nc.gpsimd.collective_compute(
                        "AllGather",
                        mybir.AluOpType.bypass,
                        replica_groups=[list(range(M))],
                        ins=[h_loc[rlo:rhi, :]],
                        outs=[h_full[M * rlo : M * rhi, :]],
                    )

            # ---------------- Phase 3: message passing + decoder ----------------
            # Software-pipelined: message matmuls of tile t are emitted before
            # decoder matmuls of tile t-1, so the PE stays busy while ACT
            # drains agg(t) into hmT(t).
            hmTs = [None] * UT
            asbs = [None] * F
            GTW = max([1] + list(S_U) + list(S_A) + list(S_B))

            ident = cpool.tile([P, P], f32, tag="ident")
            make_identity(nc, ident[:])

            def gather(t, St, boff, lo, hi, q):
                # ONE dma_gather fetches all St*128 source rows: out[p, s, :]
                # = h_full[lo + idx[s*128+p]].  The sliced source AP keeps the
                # dependency on just the all-gather chunks covering [lo, hi).
                gt_all = gpool.tile([P, GTW, HW], bf16, tag="gt")
                nc.gpsimd.dma_gather(
                    gt_all[:, :St, :],
                    h_full[lo:hi, :],
                    si_sb[:, boff * 8 : (boff + St) * 8],
                    St * P,
                    St * P,
                    HW,
                    queue_num=q,
                )
                return gt_all

            def load_wbs(boff, n):
                wbs = []
                for s in range(n):
                    wb = wpool.tile([P, P], bf16, tag="wb")
                    nc.sync.dma_start(out=wb[:], in_=wb_d[boff + s])
                    wbs.append(wb)
                return wbs

            def emit_msg_a(t):
                # primed tile, epoch A: prefix sources + self-loop, parked in
                # SBUF f32.  Runs during the all-gather tail (only needs the
                # first EPC chunks).
                psA = ps_acc.tile([P, 512], f32, tag="acc")
                St = S_A[t]
                gt_all = gather(t, St, BOFF_A[t], 0, PFX, t % 4) if St else None
                wbs = load_wbs(BOFF_A[t], St)
                wbself = wpool.tile([P, P], bf16, tag="wb")
                nc.sync.dma_start(out=wbself[:], in_=wb_d[NBLK + t])
                wbs.append(wbself)
                for c in range(4):
                    for s in range(St + 1):
                        lhsT = (
                            gt_all[:, s, c * 126 : (c + 1) * 126]
                            if s < St
                            else hsbs[t][:, c * 126 : (c + 1) * 126]
                        )
                        nc.tensor.matmul(
                            out=psA[0:126, c * P : (c + 1) * P],
                            lhsT=lhsT,
                            rhs=wbs[s][:],
                            start=(s == 0),
                            stop=(s == St),
                        )
                asb = apool.tile([P, 512], f32, tag="asb")
                asbs[t] = asb
                nc.scalar.activation(
                    out=asb[0:126, :],
                    in_=psA[0:126, :],
                    func=mybir.ActivationFunctionType.Copy,
                )

            def emit_msg_core(t, agg_ps):
                # drain agg psum -> hmT bf16 (+ bias/fill rows via DMA)
                hmT = mpool.tile([P, 512], bf16, tag="hmT")
                hmTs[t] = hmT
                # hidden unit 500 (chunk 3, row 122): decoder-bias unit
                # hidden unit 501 (chunk 3, row 123): row-mask fill unit
                # rows 124-125 are zero padding.  The ACT copy below skips
                # rows 122+ of chunk 3, so this DMA has no dependency on the
                # agg drain and can land during the message matmuls.
                nc.sync.dma_start(
                    out=hmT[122:126, 3 * P : 4 * P],
                    in_=rv_d[0:4, t * P : (t + 1) * P],
                )
                nc.scalar.activation(
                    out=hmT[0:126, 0 : 3 * P],
                    in_=agg_ps[0:126, 0 : 3 * P],
                    func=mybir.ActivationFunctionType.Copy,
                )
                nc.scalar.activation(
                    out=hmT[0:122, 3 * P : 4 * P],
                    in_=agg_ps[0:122, 3 * P : 4 * P],
                    func=mybir.ActivationFunctionType.Copy,
                )

            def emit_msg_b(t):
                # primed tile, epoch B: suffix sources, then the parked
                # epoch-A partial is added back via an exact f32 identity
                # matmul inside the same accumulation group.
                psB = ps_acc.tile([P, 512], f32, tag="acc")
                St = S_B[t]
                gt_all = gather(t, St, BOFF_B[t], PFX, M * UPC, t % 4) if St else None
                wbs = load_wbs(BOFF_B[t], St)
                for c in range(4):
                    for s in range(St):
                        nc.tensor.matmul(
                            out=psB[0:126, c * P : (c + 1) * P],
                            lhsT=gt_all[:, s, c * 126 : (c + 1) * 126],
                            rhs=wbs[s][:],
                            start=(s == 0),
                            stop=False,
                        )
                    nc.tensor.matmul(
                        out=psB[0:126, c * P : (c + 1) * P],
                        lhsT=ident[0:126, 0:126],
                        rhs=asbs[t][0:126, c * P : (c + 1) * P],
                        start=(St == 0),
                        stop=True,
                    )
                emit_msg_core(t, psB)

            def emit_msg(t):
                agg_ps = ps_acc.tile([P, 512], f32, tag="acc")
                St = S_U[t - F]
                boff = BOFF_U[t - F]
                gt_all = gather(t, St, boff, 0, M * UPC, t % 4)
                wbs = load_wbs(boff, St)
                # self-loop block: this core's own h tile straight from SBUF
                # with a diagonal weight block (no gather, no DMA).
                wbself = wpool.tile([P, P], bf16, tag="wb")
                nc.sync.dma_start(out=wbself[:], in_=wb_d[NBLK + t])
                wbs.append(wbself)
                # keep each PSUM sub-region's accumulation group contiguous:
                # interleaved start=True matmuls in one bank clobber each
                # other's accumulation state.
                for c in range(4):
                    for s in range(St + 1):
                        lhsT = (
                            gt_all[:, s, c * 126 : (c + 1) * 126]
                            if s < St
                            else hsbs[t][:, c * 126 : (c + 1) * 126]
                        )
                        nc.tensor.matmul(
                            out=agg_ps[0:126, c * P : (c + 1) * P],
                            lhsT=lhsT,
                            rhs=wbs[s][:],
                            start=(s == 0),
                            stop=(s == St),
                        )
                emit_msg_core(t, agg_ps)

            def emit_dec(t):
                hmT = hmTs[t]
                nu = UPC - t * P if t == UT - 1 else P  # 68 on the last tile
                for half in range(2):
                    # batch 6 x 500-col chunks into one SBUF row-block so the
                    # output DMA moves contiguous 6KB rows.
                    psb = opool.tile([P, 3000], f16, tag="psb")
                    for nn in range(6):
                        n = half * 6 + nn
                        p_ps = ps_dec.tile([P, 512], f32, tag="pps")
                        for c in range(4):
                            nc.tensor.matmul(
                                out=p_ps[:, :500],
                                lhsT=hmT[0:126, c * P : (c + 1) * P],
                                rhs=dec_sb[0:126, c * N_ITEMS + n * 500 : c * N_ITEMS + (n + 1) * 500],
                                start=(c == 0),
                                stop=(c == 3),
                            )
                        nc.vector.tensor_scalar(
                            out=psb[:, nn * 500 : (nn + 1) * 500],
                            in0=p_ps[:, :500],
                            scalar1=R_MAX,
                            scalar2=R_MIN,
                            op0=mybir.AluOpType.min,
                            op1=mybir.AluOpType.max,
                        )
                    nc.sync.dma_start(
                        out=out_d[t * P : t * P + nu, half * 3000 : (half + 1) * 3000],
                        in_=psb[:nu, :],
                    )

            # epoch-A sweep of the primed tiles fills the PE during the
            # all-gather tail; then the usual msg(t) / dec(t-1) interleave.
            for t in range(F):
                emit_msg_a(t)
            for t in range(UT):
                if t < F:
                    emit_msg_b(t)
                else:
                    emit_msg(t)
                if t > 0:
                    emit_dec(t - 1)
            emit_dec(UT - 1)

    nc.finalize()
    return nc


def _prep_host(x, edge_index, edge_weight, ft_n0, ft_n1, fill_const,
               enc_w, enc_b, dec_w, dec_b, conv_w):
    """All host-side preprocessing: sharding, weight prep, edge packing."""
    x = np.asarray(x, np.float32)
    src = np.asarray(edge_index[0], np.int64)
    dst = np.asarray(edge_index[1], np.int64)
    w = np.asarray(edge_weight, np.float32)
    ft_n0 = np.asarray(ft_n0)
    ft_n1 = np.asarray(ft_n1)
    fill = float(np.asarray(fill_const))
    conv = float(np.asarray(conv_w))
    enc_w = np.asarray(enc_w, np.float32)
    enc_b = np.asarray(enc_b, np.float32)
    dec_w = np.asarray(dec_w, np.float32)
    dec_b = np.asarray(dec_b, np.float32)

    rowmask = ft_n0 == 0  # rows forced to fill
    colmask = ft_n1 == 0  # cols forced to fill

    # ---- x per core, transposed to item-major tiles on host ----
    # layout: [UT, 128 (item-in-chunk p), KC, 128 (user u)] so each user
    # tile is one contiguous [128, KC*128] bf16 DMA and lhsT chunks are
    # direct column slices.
    xp = np.zeros((M, UPAD, IPAD), np.float32)
    xp[:, :UPC, :N_ITEMS] = x.reshape(M, UPC, N_ITEMS)
    xp[:, :, N_ITEMS] = 1.0  # encoder-bias input column
    xt_host = np.ascontiguousarray(
        xp.reshape(M, UT, 128, KC, 128).transpose(0, 1, 4, 3, 2)
    ).astype(_bf16).reshape(M, UPAD, KC * 128)

    # ---- encoder weights: [6016, 500] -> [128, 47*500] chunk-major ----
    ewp = np.zeros((IPAD, HIDDEN), np.float32)
    ewp[:N_ITEMS] = enc_w.T
    ewp[N_ITEMS] = enc_b
    enc_host = np.ascontiguousarray(
        ewp.reshape(KC, 128, HIDDEN).transpose(1, 0, 2).reshape(128, KC * HIDDEN)
    ).astype(_bf16)

    # ---- decoder weights with baked column mask / bias / fill units ----
    dw = dec_w.T.copy()  # [500, 6000]
    dw[:, colmask] = 0.0
    hp = np.zeros((HPAD, N_ITEMS), np.float32)
    hp[:HIDDEN] = dw
    hp[HIDDEN] = np.where(colmask, fill, dec_b)  # bias unit
    hp[HIDDEN + 1] = fill  # row-mask fill unit (all cols)
    dec_host = np.zeros((128, 4, N_ITEMS), np.float32)
    dec_host[:126] = hp.reshape(4, 126, N_ITEMS).transpose(1, 0, 2)
    dec_host = np.ascontiguousarray(dec_host.reshape(128, 4 * N_ITEMS)).astype(_bf16)

    # ---- edges: filter masked dst, fold conv_w ----
    keep = ~rowmask[dst]
    src_a = src[keep]
    dst_a = dst[keep]
    w_a = w[keep] * conv

    order = np.argsort(dst_a, kind="stable")
    src_a, dst_a, w_a = src_a[order], dst_a[order], w_a[order]

    core = dst_a // UPC
    ldst = dst_a - core * UPC
    tile_g = core * UT + ldst // 128  # global tile id (sorted ascending)
    din = (ldst % 128).astype(np.int64)

    # gather index into the PADDED all-gathered h table.
    # h_full layout after the uneven chunked all-gather: chunk j covers local
    # rows [lo_j*128, hi_j*128) of every core, concatenated core-major:
    # row = off_j + core * crows_j + (local - lo_j*128)
    src_core = src_a // UPC
    src_loc = src_a % UPC
    bounds_rows = np.array([min(b * 128, UPC) for b in CC_TILE_BOUNDS])
    starts_rows = np.concatenate([[0], bounds_rows[:-1]])
    crows = bounds_rows - starts_rows
    offs = np.concatenate([[0], np.cumsum(M * crows)[:-1]])
    cjs = np.searchsorted(bounds_rows, src_loc, side="right")
    gsrc_e = (
        offs[cjs] + src_core * crows[cjs] + (src_loc - starts_rows[cjs])
    ).astype(np.int64)

    # per-(tile, epoch) block quotas (max over cores, so the SPMD program is
    # identical on every core).  The first F_PRIME dst tiles are split into
    # epoch A (sources in the h_full prefix written by the first EPC
    # all-gather chunks, idx as-is) and epoch B (suffix sources, idx rebased)
    # so epoch A can run during the all-gather tail.
    EPC = 4
    PFX = M * int(bounds_rows[EPC - 1])
    t_of_edge = tile_g % UT
    in_prime = t_of_edge < F_PRIME
    in_sfx = gsrc_e >= PFX

    def pack(sel, rebase, min1_from=None):
        """Pack selected edges into per-tile 128-edge blocks.

        dma_gather index layout: idx j of tile t at column boff[t]*8 + j//16,
        partition j%16, replicated 8x down the 128 partitions.  Padding uses
        index 0 (gathers a real row, multiplied by weight 0).
        """
        tg = tile_g[sel]
        gi_all = gsrc_e[sel] - rebase
        dn = din[sel]
        ww = w_a[sel]
        cnt = np.bincount(tg, minlength=M * UT).reshape(M, UT)
        S_t = np.ceil(cnt.max(axis=0) / 128).astype(np.int64)
        if min1_from is not None:
            S_t[min1_from:] = np.maximum(1, S_t[min1_from:])
        boff = np.concatenate([[0], np.cumsum(S_t)[:-1]])
        nblk = int(S_t.sum())
        si_h = np.zeros((M, 128, nblk * 8), np.int16)
        wb_h = np.zeros((M, nblk, 128, 128), np.float32)
        starts = np.zeros(M * UT + 1, np.int64)
        np.cumsum(cnt.reshape(-1), out=starts[1:])
        for g in range(M * UT):
            c, t = divmod(g, UT)
            St = int(S_t[t])
            if St == 0:
                continue
            n = int(cnt[c, t])
            sl = slice(starts[g], starts[g] + n)
            cap = St * 128
            gi = np.zeros(cap, np.int64)
            wi = np.zeros(cap, np.float32)
            di = np.zeros(cap, np.int64)
            gi[:n] = gi_all[sl]
            wi[:n] = ww[sl]
            di[:n] = dn[sl]
            b0 = int(boff[t])
            wrap = gi.astype(np.int16).reshape(-1, 16).T  # [16, S*8]
            si_h[c, :, b0 * 8 : (b0 + St) * 8] = np.tile(wrap, (8, 1))
            for q in range(St):
                blk = slice(q * 128, (q + 1) * 128)
                wb_h[c, b0 + q][np.arange(128), di[blk]] = wi[blk]
        return S_t, si_h, wb_h

    SA_f, si_A, wb_A = pack(in_prime & ~in_sfx, 0)
    SB_f, si_B, wb_B = pack(in_prime & in_sfx, PFX)
    SU_f, si_U, wb_U = pack(~in_prime, 0, min1_from=F_PRIME)
    S_A = tuple(int(v) for v in SA_f[:F_PRIME])
    S_B = tuple(int(v) for v in SB_f[:F_PRIME])
    S_U = tuple(int(v) for v in SU_f[F_PRIME:])
    si_host = np.concatenate([si_A, si_B, si_U], axis=2)
    wblk_host = np.concatenate([wb_A, wb_B, wb_U], axis=1)
    # diagonal self-loop weight blocks, appended after the gather blocks:
    # block NBLK + t applies (1-conv)*live(d) to the SBUF h tile t.
    lv = np.zeros((M, UPAD), np.float32)
    lv[:, :UPC] = (~rowmask).reshape(M, UPC).astype(np.float32) * (1.0 - conv)
    wself = np.zeros((M, UT, 128, 128), np.float32)
    di128 = np.arange(128)
    for t in range(UT):
        wself[:, t, di128, di128] = lv[:, t * 128 : (t + 1) * 128]
    wblk_host = np.concatenate([wblk_host, wself], axis=1).astype(_bf16)

    # ---- row vectors: bias-unit coeff and row-mask coeff per padded user
    # (rows 2-3 are zero fillers for hmT pad rows 124-125) ----
    rv = np.zeros((M, 4, UPAD), np.float32)
    rm = rowmask.reshape(M, UPC)
    rv[:, 0, :UPC] = (~rm).astype(np.float32)  # bias unit on for live rows
    rv[:, 1, :UPC] = rm.astype(np.float32)     # fill unit on for masked rows
    rv_host = rv.astype(_bf16)

    in_maps = []
    for c in range(M):
        in_maps.append(
            {
                "x": xt_host[c],
                "encw": enc_host,
                "decw": dec_host,
                "sidx": si_host[c],
                "wblk": wblk_host[c],
                "rowvec": rv_host[c],
            }
        )
    return S_A, S_B, S_U, in_maps


def _install_ntff_hook_shim():
    """The agent image's antenv lacks axon_hooks; synthesize it so
    run_bass_kernel_spmd(trace=True) can capture NTFF profiles."""
    import types

    if "antenv.axon_hooks" in sys.modules:
        return
    try:
        from trn_agent_boot.trn_boot import _ntff_profile_via_ctypes
    except ImportError:
        return
    hook = _ntff_profile_via_ctypes("/opt/axon/libaxon_pjrt.so")
    mod = types.ModuleType("antenv.axon_hooks")
    mod._hook = hook
    mod.set_axon_ntff_profile_hook = lambda h: setattr(mod, "_hook", h)
    mod.get_axon_ntff_profile_hook = lambda: mod._hook
    sys.modules["antenv.axon_hooks"] = mod
    try:
        import antenv

        antenv.axon_hooks = mod
    except ImportError:
        pass


LAST_EXEC_NS = None
LAST_RESULTS = None


def kernel(x, edge_index, edge_weight, ft_n0, ft_n1, fill_const,
           enc_w, enc_b, dec_w, dec_b, conv_w):
    global LAST_EXEC_NS, LAST_RESULTS
    from concourse.bass_utils import run_bass_kernel_spmd

    S_A, S_B, S_U, in_maps = _prep_host(
        x, edge_index, edge_weight, ft_n0, ft_n1, fill_const,
        enc_w, enc_b, dec_w, dec_b, conv_w,
    )

    key = (S_A, S_B, S_U)
    if key not in _PROGRAM_CACHE:
        _PROGRAM_CACHE[key] = _build_program(S_A, S_B, S_U)
    nc = _PROGRAM_CACHE[key]

    trace = os.environ.get("KERNEL_TRACE", "0") == "1"
    tmpdir = os.environ.get("KERNEL_TRACE_DIR") or None
    if trace:
        _install_ntff_hook_shim()
    res = run_bass_kernel_spmd(
        nc,
        in_maps,
        core_ids=list(range(M)),
        trace=trace,
        tmpdir=tmpdir,
    )
    LAST_EXEC_NS = res.exec_time_ns
    LAST_RESULTS = res
    out = np.concatenate([res.results[c]["out"] for c in range(M)], axis=0)
    return np.ascontiguousarray(out.astype(np.float32))
